# revision 36
# baseline (speedup 1.0000x reference)
"""DTCWT 3-level inverse on 8 Trainium2 NeuronCores.

Every filtering stage is a banded matmul on the tensor engine in fp16
(1 col/cycle at any N; ~1e-3 pipeline rel err, well under the 2e-2 gate).

All stages use "data as lhsT" mode: matmul(out, lhsT=data[K=h, M=w],
rhs=mat[K=h, N=h_out]) contracts over the partition dim of the data and
yields the filtered image TRANSPOSED ([w, h_out]); column and row stages
then alternate orientation naturally with zero explicit transposes.

The c2q band construction is folded into the matrices. Column stages are
band-blocked so that the FULL contraction (complex x pair x band-support)
of each output block fits in <=128 partitions: the real/imag parts are
K-stacked into the partition dim on the host (instead of stride-2 lhsT
column slices), and each h_out block needs ONE single-pass matmul. PSUM
outputs use an (h_out, w-parity)-interleaved column layout so each block
writes one contiguous dst; downstream row stages read stride-2 slices.
This roughly halves the streamed matrix columns of every column stage
vs. 2-pass full-width matmuls.

Inputs are cast to fp16 on the host (halves HBM traffic); each group's
band blocks ship as one contiguous DRAM tensor (uniform rows per block
family) loading with a single DMA trigger. All matrices ship in one
packed [128, N] dram tensor.

Sharding: pure data parallel over batch N (8 cores x 16 channels each).
"""
import sys

for _p in ('/opt/trn_rl_repo',):
    if _p not in sys.path:
        sys.path.append(_p)

import numpy as np
import concourse.bass as bass
import concourse.mybir as mybir
from concourse.tile import TileContext
from concourse.bass_utils import run_bass_kernel_spmd

SQRT_HALF = 0.7071067811865476
N_CORES = 8
IMGS_PER_CORE = 16
GROUPS = 4
F32 = mybir.dt.float32
F16 = mybir.dt.float16

# Synthesis filters (same constants as the reference; the runtime-passed
# copies are used for the actual matrix values, these fix the structure).
G0O = np.array([0.35355339059327, 0.70710678118655, 0.35355339059327],
               dtype=np.float64)
G1O = np.array([-0.08838834764832, -0.17677669529664, 0.53033008588991,
                -0.17677669529664, -0.08838834764832], dtype=np.float64)
G0A = np.array([0.03516384, 0.0, -0.08832942, 0.23389032, 0.76027237,
                0.5875183, 0.0, -0.11430184, 0.0, 0.0], dtype=np.float64)
G0B = G0A[::-1].copy()
_alt = (-1.0) ** np.arange(10)
G1A = G0B * _alt
G1B = G1A[::-1].copy()


# ---------------------------------------------------------------------------
# Host-side matrix construction (numpy, float64)
# ---------------------------------------------------------------------------
def _conv_rows_valid(x, h):
    hr = h[::-1]
    taps = h.shape[0]
    n = x.shape[-2] - taps + 1
    out = hr[0] * x[..., 0:n, :]
    for k in range(1, taps):
        out = out + hr[k] * x[..., k:k + n, :]
    return out


def _pad_rows_symmetric(x, m):
    pad = [(0, 0)] * (x.ndim - 2) + [(m, m), (0, 0)]
    return np.pad(x, pad, mode='symmetric')


def _colfilter(x, h):
    return _conv_rows_valid(_pad_rows_symmetric(x, h.shape[0] // 2), h)


def _colifilt(x, ha, hb, highpass):
    m = ha.shape[0]
    m2 = m // 2
    r = x.shape[-2]
    xp = _pad_rows_symmetric(x, m2)
    xe = xp[..., 1:r + m - 2:2, :]
    xo = xp[..., 2:r + m - 1:2, :]
    xa, xb = (xe, xo) if highpass else (xo, xe)
    hao, hae = ha[0::2], ha[1::2]
    hbo, hbe = hb[0::2], hb[1::2]
    y0 = _conv_rows_valid(xb, hao)
    y1 = _conv_rows_valid(xa, hbo)
    y2 = _conv_rows_valid(xb, hae)
    y3 = _conv_rows_valid(xa, hbe)
    y = np.stack([y0, y1, y2, y3], axis=-2)
    return y.reshape(y.shape[:-3] + (2 * r, y.shape[-1]))


def _op_matrix(op, n):
    """M[h_in, h_out] with out[h_out, w] = sum_h M[h, h_out] x[h, w]."""
    return np.ascontiguousarray(op(np.eye(n, dtype=np.float64)).T)


def _base_mats(g0o, g1o, g0a, g0b, g1a, g1b):
    """Per-level even/odd polyphase matrices (float64)."""
    s = SQRT_HALF
    b = {}
    # L3 (R=64) / L2 (R=128) quarter-shift upsampling matrices
    for R, tag in ((64, '3'), (128, '2')):
        Mlo = _op_matrix(lambda x: _colifilt(x, g0b, g0a, False), R)
        Mhi = _op_matrix(lambda x: _colifilt(x, g1b, g1a, True), R)
        b[f'Mlo{tag}'] = Mlo
        b[f'Mhi{tag}'] = Mhi
        b[f'Mle{tag}'], b[f'Mloo{tag}'] = s * Mlo[0::2], s * Mlo[1::2]
        b[f'Mhe{tag}'], b[f'Mho{tag}'] = s * Mhi[0::2], s * Mhi[1::2]
    # L1 (size-preserving colfilter, n=256)
    A_lo = _op_matrix(lambda x: _colfilter(x, g0o), 256)
    A_hi = _op_matrix(lambda x: _colfilter(x, g1o), 256)
    b['A_lo'], b['A_hi'] = A_lo, A_hi
    b['Ale'], b['Alo'] = s * A_lo[0::2], s * A_lo[1::2]
    b['Ahe'], b['Aho'] = s * A_hi[0::2], s * A_hi[1::2]
    return b


_BASE = _base_mats(G0O, G1O, G0A, G0B, G1A, G1B)


def _support(mats, j0, j1):
    """Union nonzero-row range of mats' columns [j0, j1)."""
    acc = 0
    for m in mats:
        acc = acc + np.abs(m[:, j0:j1]).max(axis=1)
    rows = np.nonzero(acc > 1e-12)[0]
    return int(rows.min()), int(rows.max()) + 1


def _uniform_supports(mats, blocks, nmax):
    """Per-block row ranges, padded to a uniform length (within [0, nmax))."""
    sups = [_support(mats, j0, j1) for j0, j1 in blocks]
    nr = max(r1 - r0 for r0, r1 in sups)
    out = []
    for r0, r1 in sups:
        r1 = min(nmax, r0 + nr)
        r0 = r1 - nr
        out.append((r0, r1))
    return out, nr


# Block tables
L1B = tuple((j, min(j + 43, 256)) for j in range(0, 256, 43))   # 6 blocks
L2HB = tuple((j, j + 64) for j in range(0, 256, 64))            # 4 blocks
L2QB = tuple((j, j + 32) for j in range(0, 256, 32))            # 8 blocks

L1H_SUP, NRH = _uniform_supports((_BASE['Ahe'], _BASE['Aho']), L1B, 128)
L1L_SUP, NRL = _uniform_supports((_BASE['Ale'], _BASE['Alo']), L1B, 128)
L2H_SUP, NR2H = _uniform_supports((_BASE['Mhe2'], _BASE['Mho2']), L2HB, 64)
L2Q_SUP, NRQ = _uniform_supports(
    (_BASE['Mle2'], _BASE['Mloo2'], _BASE['Mhe2'], _BASE['Mho2']), L2QB, 64)
assert 4 * NRH <= 128 and 4 * NRL <= 128
assert 4 * NR2H <= 128 and 8 * NRQ <= 128


def _wblk(Me, Mo, r0, r1, j0, j1):
    """Pair-band block matrix [4nr, 2w]: rows (c{R,I}, p{1,2}, band),
    cols (h_out, w-parity) interleaved."""
    combos = ((Me, -Mo), (Me, Mo), (Mo, Me), (-Mo, Me))
    nr, w = r1 - r0, j1 - j0
    W = np.zeros((4 * nr, 2 * w))
    for k, (E, O) in enumerate(combos):
        W[k * nr:(k + 1) * nr, 0::2] = E[r0:r1, j0:j1]
        W[k * nr:(k + 1) * nr, 1::2] = O[r0:r1, j0:j1]
    return W


def _qblk(Mle, Mlo, Mhe, Mho, r0, r1, j0, j1):
    """Quad-band block [8nr, 2w]: rows (c, orient{2,3,1,4}, band)."""
    combos = ((Mle, -Mlo), (Mle, Mlo), (Mhe, -Mho), (Mhe, Mho),
              (Mlo, Mle), (-Mlo, Mle), (Mho, Mhe), (-Mho, Mhe))
    nr, w = r1 - r0, j1 - j0
    W = np.zeros((8 * nr, 2 * w))
    for k, (E, O) in enumerate(combos):
        W[k * nr:(k + 1) * nr, 0::2] = E[r0:r1, j0:j1]
        W[k * nr:(k + 1) * nr, 1::2] = O[r0:r1, j0:j1]
    return W


def build_matrices(g0o, g1o, g0a, g0b, g1a, g1b):
    """All device matrices as {name: fp16 ndarray}."""
    b = _base_mats(np.asarray(g0o, np.float64), np.asarray(g1o, np.float64),
                   np.asarray(g0a, np.float64), np.asarray(g0b, np.float64),
                   np.asarray(g1a, np.float64), np.asarray(g1b, np.float64))
    hs, vs = np.hstack, np.vstack
    out = {}

    # ---- L3 ----
    out['M3_lo'] = b['Mlo3']                                     # [64, 128]
    Me3, Mo3 = b['Mhe3'], b['Mho3']                              # [32, 128]
    out['L3n'] = vs([hs([Me3, -Mo3]), hs([Me3, Mo3]),
                     hs([Mo3, Me3]), hs([-Mo3, Me3])])           # [128, 256]
    Mel3, Mol3 = b['Mle3'], b['Mloo3']
    L3_lo_R = hs([vs([Mel3, Mel3]), vs([-Mol3, Mol3])])
    L3_lo_I = hs([vs([Mol3, -Mol3]), vs([Mel3, Mel3])])
    L3_hi_R = hs([vs([Me3, Me3]), vs([-Mo3, Mo3])])
    L3_hi_I = hs([vs([Mo3, -Mo3]), vs([Me3, Me3])])
    out['L3_q_R'] = vs([L3_lo_R, L3_hi_R])                       # [128, 256]
    out['L3_q_I'] = vs([L3_lo_I, L3_hi_I])
    out['R3_A'] = vs([b['Mlo3'][0::2], b['Mlo3'][1::2], b['Mlo3']])
    out['R3_B'] = vs([b['Mhi3'][0::2], b['Mhi3'][1::2]])         # [64, 128]

    # ---- L2 ----
    out['M2_lo'] = b['Mlo2']                                     # [128, 256]
    for i, ((j0, j1), (r0, r1)) in enumerate(zip(L2HB, L2H_SUP)):
        out[f'Lh{i}'] = _wblk(b['Mhe2'], b['Mho2'], r0, r1, j0, j1)
    for i, ((j0, j1), (r0, r1)) in enumerate(zip(L2QB, L2Q_SUP)):
        out[f'Qn{i}'] = _qblk(b['Mle2'], b['Mloo2'], b['Mhe2'], b['Mho2'],
                              r0, r1, j0, j1)
    out['R2_E'] = vs([b['Mlo2'][0::2], b['Mhi2'][0::2]])         # [128, 256]
    out['R2_O'] = vs([b['Mlo2'][1::2], b['Mhi2'][1::2]])

    # ---- L1 ----
    A_lo = b['A_lo']
    out['A0'] = A_lo[0:128, 0:126]
    out['A1a'] = A_lo[64:128, 126:130]
    out['A1b'] = A_lo[128:192, 126:130]
    out['A2'] = A_lo[128:256, 130:256]
    for i, (j0, j1) in enumerate(L1B):
        r0, r1 = L1H_SUP[i]
        out[f'Wh{i}'] = _wblk(b['Ahe'], b['Aho'], r0, r1, j0, j1)
        r0, r1 = L1L_SUP[i]
        out[f'Wl{i}'] = _wblk(b['Ale'], b['Alo'], r0, r1, j0, j1)
    out['Be1_lo'], out['Bo1_lo'] = A_lo[0::2], A_lo[1::2]        # [128, 256]
    out['Be1_hi'], out['Bo1_hi'] = b['A_hi'][0::2], b['A_hi'][1::2]
    return {k: np.ascontiguousarray(v, np.float16) for k, v in out.items()}


_L3_KEYS = ('M3_lo', 'L3n', 'L3_q_R', 'L3_q_I', 'R3_A', 'R3_B')

# name -> (rows, cols, rowbase); order = usage order (chunk streaming)
MAT_SHAPES = {
    'M3_lo': (64, 128, 0),
    'L3n': (128, 256, 0),
    'L3_q_R': (128, 256, 0), 'L3_q_I': (128, 256, 0),
    'R3_A': (128, 128, 0), 'R3_B': (64, 128, 0),
    'M2_lo': (128, 256, 0),
}
for _i in range(4):
    MAT_SHAPES[f'Lh{_i}'] = (4 * NR2H, 128, 0)
for _i in range(8):
    MAT_SHAPES[f'Qn{_i}'] = (8 * NRQ, 64, 0)
MAT_SHAPES.update({
    'R2_E': (128, 256, 0), 'R2_O': (128, 256, 0),
    'A0': (128, 126, 0), 'A1a': (64, 4, 64), 'A1b': (64, 4, 0),
    'A2': (128, 126, 0),
})
for _i in range(6):
    MAT_SHAPES[f'Wh{_i}'] = (4 * NRH, 2 * (L1B[_i][1] - L1B[_i][0]), 0)
for _i in range(6):
    MAT_SHAPES[f'Wl{_i}'] = (4 * NRL, 2 * (L1B[_i][1] - L1B[_i][0]), 0)
MAT_SHAPES.update({
    'Be1_lo': (128, 256, 0), 'Bo1_lo': (128, 256, 0),
    'Be1_hi': (128, 256, 0), 'Bo1_hi': (128, 256, 0),
})

MAT_OFF = {}
MATS_COLS = 0
for _k, (_kk, _nn, _rb) in MAT_SHAPES.items():
    MAT_OFF[_k] = MATS_COLS
    MATS_COLS += _nn

# L3 matrices load first as small per-matrix DMAs (trimmed to K rows);
# the rest stream in three usage-ordered chunks: L2 mats (S1/S2), L1
# column mats (S3/S4), L1 row mats (S5).
MAT_CHUNKS = [
    (MAT_OFF['M2_lo'], MAT_OFF['A0']),
    (MAT_OFF['A0'], MAT_OFF['Be1_lo']),
    (MAT_OFF['Be1_lo'], MATS_COLS),
]


def pack_matrices(mats):
    big = np.zeros((128, MATS_COLS), np.float16)
    for k, m in mats.items():
        kk, nn, rb = MAT_SHAPES[k]
        assert m.shape == (kk, nn), (k, m.shape)
        big[rb:rb + kk, MAT_OFF[k]:MAT_OFF[k] + nn] = m
    return big


# ---------------------------------------------------------------------------
# Input packing: per-core, per-group tiles, contiguous in DRAM.
# Band-block tiles K-stack (complex, pair/orient, band-rows) into the
# partition dim; cols are (img, block, w) so per-(img, block) lhsT slices
# are contiguous and the tile loads with one (or two) DMA triggers.
# ---------------------------------------------------------------------------
def _pack_pair_blocks(a6, sups, w):
    """a6: [16, 2(pair), H, w, 2] -> [G, 4nr, 4imgs * nblk * w]."""
    H = a6.shape[2]
    a = a6.reshape(GROUPS, 4, 2, H, w, 2)
    parts = [a[:, :, :, r0:r1].transpose(0, 5, 2, 3, 1, 4)
             for (r0, r1) in sups]                  # [G, 2c, 2p, nr, 4i, w]
    stk = np.stack(parts, axis=5)                   # [G, 2c, 2p, nr, 4i, B, w]
    nr = sups[0][1] - sups[0][0]
    return np.ascontiguousarray(stk.reshape(GROUPS, 4 * nr, -1))


def _pack_quad_blocks(yh1, sups):
    a = yh1[:, (2, 3, 1, 4)]                        # [16, 4o, 64, 64, 2]
    a = a.reshape(GROUPS, 4, 4, 64, 64, 2)
    parts = [a[:, :, :, r0:r1].transpose(0, 5, 2, 3, 1, 4)
             for (r0, r1) in sups]                  # [G, 2c, 4o, nr, 4i, 64]
    stk = np.stack(parts, axis=5)
    nr = sups[0][1] - sups[0][0]
    return np.ascontiguousarray(stk.reshape(GROUPS, 8 * nr, -1))


def pack_inputs(yl, yh0, yh1, yh2):
    t = {}
    t['z3p'] = np.ascontiguousarray(
        yl.reshape(GROUPS, 4, 64, 64).transpose(0, 2, 1, 3)
        .reshape(GROUPS, 64, 256))
    # lh3: rows (c, p, h32), cols (img, w32)
    a = yh2[:, (0, 5)].reshape(GROUPS, 4, 2, 32, 32, 2)
    t['lh3p'] = np.ascontiguousarray(
        a.transpose(0, 5, 2, 3, 1, 4).reshape(GROUPS, 128, 128))
    # q3: quad rows (band(hl,hh), p, h32), cols (img, w.c interleaved)
    def pg(y, osel):
        a = y[:, osel]
        h = a.shape[2]
        a = a.reshape(GROUPS, 4, 2, h, -1)
        return np.ascontiguousarray(
            a.transpose(0, 2, 3, 1, 4).reshape(GROUPS, 2 * h, -1))
    t['q3p'] = np.ascontiguousarray(np.concatenate(
        [pg(yh2, slice(2, 4)), pg(yh2, slice(1, 5, 3))], axis=1))
    # L2
    t['lh2p'] = _pack_pair_blocks(yh1[:, (0, 5)], L2H_SUP, 64)
    t['qbp'] = _pack_quad_blocks(yh1, L2Q_SUP)
    # L1
    t['t05p'] = _pack_pair_blocks(yh0[:, (0, 5)], L1H_SUP, 128)
    t['t23p'] = _pack_pair_blocks(yh0[:, (2, 3)], L1L_SUP, 128)
    t['t14p'] = _pack_pair_blocks(yh0[:, (1, 4)], L1H_SUP, 128)
    return t


IN_SHAPES = {
    'z3p': (GROUPS, 64, 256),
    'lh3p': (GROUPS, 128, 128),
    'q3p': (GROUPS, 128, 256),
    'lh2p': (GROUPS, 4 * NR2H, 4 * 4 * 64),
    'qbp': (GROUPS, 8 * NRQ, 4 * 8 * 64),
    't05p': (GROUPS, 4 * NRH, 4 * 6 * 128),
    't23p': (GROUPS, 4 * NRL, 4 * 6 * 128),
    't14p': (GROUPS, 4 * NRH, 4 * 6 * 128),
}


# ---------------------------------------------------------------------------
# Bass kernel
# ---------------------------------------------------------------------------
def split_excess_waits(nc, max_waits=1):
    """walrus CTRL codegen allows only one sem wait per instruction; move
    excess waits onto NoOps inserted just before the offending instruction."""
    ctr = 0
    for fn in nc.m.functions:
        for bb in fn.blocks:
            new_list = []
            for inst in bb.instructions:
                si = inst.sync_info
                if si is not None and si.on_wait and len(si.on_wait) > max_waits:
                    waits = list(si.on_wait)
                    keep, extra = waits[:max_waits], waits[max_waits:]
                    for i in range(0, len(extra), max_waits):
                        nop = mybir.InstNoOp(
                            name=f"wait_split_{ctr}", ins=[], outs=[])
                        ctr += 1
                        nop.engine = inst.engine
                        nop.sync_info = mybir.SyncInfo(
                            on_wait=extra[i:i + max_waits], on_update=[])
                        nc.register_instruction(nop)
                        new_list.append(nop)
                    inst.sync_info = mybir.SyncInfo(
                        on_wait=keep,
                        on_update=list(si.on_update) if si.on_update else [])
                new_list.append(inst)
            bb.instructions[:] = new_list
    return ctr


def build_nc():
    nc = bass.Bass()
    in_d = {k: nc.dram_tensor(k, list(shp), F16, kind="ExternalInput")
            for k, shp in IN_SHAPES.items()}
    out_d = nc.dram_tensor("out", [IMGS_PER_CORE, 256, 256], F16,
                           kind="ExternalOutput")
    mat_d = nc.dram_tensor("mats", [128, MATS_COLS], F16,
                           kind="ExternalInput")

    with TileContext(nc) as tc:
        with tc.tile_pool(name="mats", bufs=1) as matpool, \
             tc.tile_pool(name="data", bufs=4) as datapool, \
             tc.tile_pool(name="mid", bufs=6) as midpool, \
             tc.tile_pool(name="z2p", bufs=16) as z2pool, \
             tc.tile_pool(name="outp", bufs=4) as outpool, \
             tc.tile_pool(name="ps3", bufs=2, space="PSUM") as ps3pool, \
             tc.tile_pool(name="ring", bufs=6, space="PSUM") as ringpool:

            matT = matpool.tile([128, MATS_COLS], F16, tag="mats")
            for k in _L3_KEYS:
                kk, nn, rb = MAT_SHAPES[k]
                off = MAT_OFF[k]
                nc.scalar.dma_start(
                    out=matT[rb:rb + kk, off:off + nn],
                    in_=mat_d[rb:rb + kk, off:off + nn])

            def M(k, j0=None, j1=None):
                kk, nn, rb = MAT_SHAPES[k]
                off = MAT_OFF[k]
                a = off if j0 is None else off + j0
                bb = off + nn if j1 is None else off + j1
                return matT[rb:rb + kk, a:bb]

            def mm(out_ap, lhsT, rhs_ap, start, stop):
                nc.tensor.matmul(out_ap, lhsT, rhs_ap,
                                 start=start, stop=stop)

            _dma_engs = (nc.sync, nc.gpsimd)
            _eng_ctr = [0]

            def _load_keys(g, t, keys, eng=None):
                """keys: 'z3' etc; 't05:1'..':4' loads one image-quarter of
                the columns, ':12'/':34' loads a half."""
                for key in keys:
                    base, part = key, None
                    if ':' in key:
                        base, part = key.split(':')
                    dk = base + 'p'
                    shp = IN_SHAPES[dk]
                    tl = t.get(base)
                    if tl is None:
                        tl = datapool.tile([shp[1], shp[2]], F16,
                                           name=f"{base}_{g}", tag=base)
                        t[base] = tl
                    if eng is None:
                        eng = _dma_engs[_eng_ctr[0] % 2]
                        _eng_ctr[0] += 1
                    if part is None:
                        eng.dma_start(out=tl[:], in_=in_d[dk][g])
                    else:
                        qw = shp[2] // 4
                        c0 = (int(part[0]) - 1) * qw
                        c1 = c0 + qw * len(part)
                        eng.dma_start(out=tl[:, c0:c1],
                                      in_=in_d[dk][g, :, c0:c1])

            def _chunks(i0, i1):
                for i in range(i0, min(i1, len(MAT_CHUNKS))):
                    c0, c1 = MAT_CHUNKS[i]
                    _dma_engs[i % 2].dma_start(out=matT[:, c0:c1],
                                               in_=mat_d[:, c0:c1])

            def do_L3(t_in, q):
                def R(t, w):
                    return t[:, q * w:q * w + w:2]

                def I(t, w):
                    return t[:, q * w + 1:q * w + w:2]

                z3, lh3, q3 = t_in['z3'], t_in['lh3'], t_in['q3']
                # psA [128, 512] (1 bank):
                #   cols 0:128   y1b_e [0:32], y1b_o [32:64], y1z [64:128]
                #   cols 128:256 y2b_e [0:32], y2b_o [32:64]
                #   cols 256:384 z2 [0:128]
                psA = ps3pool.tile([128, 512], F32, tag="psA")
                mm(psA[64:128, 0:128], z3[:, q * 64:(q + 1) * 64],
                   M('M3_lo'), True, True)
                lq = lh3[:, q * 32:(q + 1) * 32]
                mm(psA[0:32, 0:128], lq, M('L3n', 0, 128), True, True)
                mm(psA[32:64, 0:128], lq, M('L3n', 128, 256), True, True)
                mm(psA[0:32, 128:256], R(q3, 64),
                   M('L3_q_R', 0, 128), True, False)
                mm(psA[0:32, 128:256], I(q3, 64),
                   M('L3_q_I', 0, 128), False, True)
                mm(psA[32:64, 128:256], R(q3, 64),
                   M('L3_q_R', 128, 256), True, False)
                mm(psA[32:64, 128:256], I(q3, 64),
                   M('L3_q_I', 128, 256), False, True)

                rowA_s = midpool.tile([128, 128], F16, tag="rowA3")
                nc.scalar.copy(rowA_s[:], psA[:, 0:128])
                rowB_s = midpool.tile([64, 128], F16, tag="rowB3")
                nc.vector.tensor_copy(out=rowB_s[:],
                                      in_=psA[0:64, 128:256])

                mm(psA[0:128, 256:384], rowA_s[:], M('R3_A'),
                   True, False)
                mm(psA[0:128, 256:384], rowB_s[:], M('R3_B'),
                   False, True)
                z2_s = z2pool.tile([128, 128], F16, tag="z2")
                nc.scalar.copy(z2_s[:], psA[:, 256:384])
                return z2_s

            def ring(nm, img):
                return ringpool.tile([128, 512], F32,
                                     name=f"{nm}_{img}", tag="ring")

            # ---- L2/L1 sub-stages, software-pipelined two images deep ----
            def S1(img, t_in, q, z2_s, st):
                """L2 column stage + drain copies. psCb cols (h, ph)."""
                lh2, qb = t_in['lh2'], t_in['qb']
                psCy = ring("psCy", img)
                psCb = ring("psCb", img)
                mm(psCy[:, 0:256], z2_s[:], M('M2_lo'), True, True)
                for b, (j0, j1) in enumerate(L2HB):
                    c = (q * 4 + b) * 64
                    mm(psCb[0:64, 2 * j0:2 * j1], lh2[:, c:c + 64],
                       M(f'Lh{b}'), True, True)
                for b, (j0, j1) in enumerate(L2QB):
                    c = (q * 8 + b) * 64
                    mm(psCb[64:128, 2 * j0:2 * j1], qb[:, c:c + 64],
                       M(f'Qn{b}'), True, True)
                st['y1zT2'] = midpool.tile([128, 256], F16,
                                           name=f"y1zT2_{img}", tag="y1zT2")
                nc.vector.tensor_copy(out=st['y1zT2'][:], in_=psCy[:, 0:256])
                st['row2'] = midpool.tile([128, 512], F16,
                                          name=f"row2_{img}", tag="row2")
                nc.scalar.copy(st['row2'][:], psCb[:])

            def S2(img, st):
                """L2 row stage -> z1 (cols (m, w))."""
                p2r = ring("p2r", img)
                for m in range(2):
                    zc = p2r[:, m * 256:(m + 1) * 256]
                    mm(zc, st['y1zT2'][:, m * 128:(m + 1) * 128],
                       M('M2_lo'), True, False)
                    mm(zc, st['row2'][:, m * 256:(m + 1) * 256:2],
                       M('R2_E'), False, False)
                    mm(zc, st['row2'][:, m * 256 + 1:(m + 1) * 256:2],
                       M('R2_O'), False, True)
                st['z1'] = midpool.tile([128, 512], F16,
                                        name=f"z1_{img}", tag="z1")
                # scalar, not vector: in the vector queue this lands behind
                # y2b1(i-1) and gates S3's first LDWEIGHTS every image.
                nc.scalar.copy(st['z1'][:], p2r[:])

            def S3(img, t_in, q, st):
                """L1 phase A: y1 = y1z + y1b in (h, ph)-interleaved PSUM."""
                z1_s = st['z1']
                t05 = t_in['t05']
                # p1y first: it depends only on t05; the z1-dependent p1z
                # block then starts after scalar's z1 drain has landed.
                p1y = ring("p1y", img)
                for b, (j0, j1) in enumerate(L1B):
                    c = (q * 6 + b) * 128
                    mm(p1y[:, 2 * j0:2 * j1], t05[:, c:c + 128],
                       M(f'Wh{b}'), True, True)
                p1z = ring("p1z", img)
                for ph in range(2):
                    mm(p1z[:, ph:252:2], z1_s[:, ph:256:2],
                       M('A0'), True, True)
                    mid = p1z[:, 252 + ph:260:2]
                    mm(mid, z1_s[64:128, ph:256:2], M('A1a'), True, False)
                    mm(mid, z1_s[0:64, 256 + ph:512:2], M('A1b'), False, True)
                    mm(p1z[:, 260 + ph:512:2], z1_s[:, 256 + ph:512:2],
                       M('A2'), True, True)
                zsb = midpool.tile([128, 512], F16,
                                   name=f"zsb_{img}", tag="zsb")
                nc.scalar.copy(zsb[:], p1z[:])
                st['y1'] = midpool.tile([128, 512], F16,
                                        name=f"y1_{img}", tag="y1")
                nc.vector.tensor_add(out=st['y1'][:], in0=p1y[:],
                                     in1=zsb[:])

            def S4(img, t_in, q, st):
                """L1 phase B: y2b (banded blocks, single pass each)."""
                t23, t14 = t_in['t23'], t_in['t14']
                p1b = ring("p1b", img)
                for b, (j0, j1) in enumerate(L1B):
                    c = (q * 6 + b) * 128
                    dst = p1b[:, 2 * j0:2 * j1]
                    mm(dst, t23[:, c:c + 128], M(f'Wl{b}'), True, False)
                    mm(dst, t14[:, c:c + 128], M(f'Wh{b}'), False, True)
                st['y2b1'] = midpool.tile([128, 512], F16,
                                          name=f"y2b1_{img}", tag="y2b1")
                # off scalar: its queue (row2 ahead of this) otherwise
                # gates S5's first LDWEIGHTS every image.
                nc.vector.tensor_copy(out=st['y2b1'][:], in_=p1b[:])

            def S5(img, st):
                """L1 row stage -> out stores."""
                p1r = ring("p1r", img)
                for m in range(2):
                    oc = p1r[:, m * 256:(m + 1) * 256]
                    me = slice(m * 256, (m + 1) * 256, 2)
                    mo = slice(m * 256 + 1, (m + 1) * 256, 2)
                    mm(oc, st['y1'][:, me], M('Be1_lo'), True, False)
                    mm(oc, st['y1'][:, mo], M('Bo1_lo'), False, False)
                    mm(oc, st['y2b1'][:, me], M('Be1_hi'), False, False)
                    mm(oc, st['y2b1'][:, mo], M('Bo1_hi'), False, True)
                    ot = outpool.tile([128, 256], F16, tag=f"out_m{m}")
                    nc.vector.tensor_copy(out=ot[:], in_=oc)
                    # scalar for the LAST image's m=1 only: parallel with
                    # sync's m=0 trigger at the pipeline tail; everywhere
                    # else a scalar trigger would delay the tight
                    # zsb/row2/z1 copy chain.
                    eng = nc.scalar if (m == 1 and img == IMGS_PER_CORE - 1) \
                        else nc.sync
                    eng.dma_start(
                        out=out_d[img, m * 128:(m + 1) * 128, :],
                        in_=ot[:])

            # Loads in image-0 critical-path order: group 0's L3 tiles,
            # then the mats + tiles each S-stage of image 0 needs, in
            # stage order (t-tiles split so images 0-1's halves land
            # first). Groups 2/3 load late; their eager-L3 batches are
            # interleaved into the S-loop.
            groups = {g: {} for g in range(GROUPS)}
            _load_keys(0, groups[0], ('z3', 'lh3', 'q3'))
            _load_keys(1, groups[1], ('z3', 'lh3', 'q3'))
            _chunks(0, 1)
            _load_keys(0, groups[0], ('lh2', 'qb'))
            _chunks(1, 2)
            _load_keys(0, groups[0], ('t05:12', 't23:12', 't14:12'))
            _chunks(2, 3)
            _load_keys(0, groups[0], ('t05:34', 't23:34', 't14:34'))
            _load_keys(1, groups[1], ('lh2', 'qb', 't05', 't23', 't14'))
            for g in (2, 3):
                _load_keys(g, groups[g], ('z3', 'lh3', 'q3', 'lh2', 'qb',
                                          't05', 't23', 't14'))

            z2s = {}

            def l3_batch(g):
                for q in range(4):
                    z2s[4 * g + q] = do_L3(groups[g], q)

            l3_batch(0)
            l3_batch(1)

            def GQ(img):
                return groups[img // 4], img % 4

            sts = {}
            sts[0] = {}
            g0, q0 = GQ(0)
            S1(0, g0, q0, z2s.pop(0), sts[0])
            S2(0, sts[0])
            S3(0, g0, q0, sts[0])
            for i in range(1, IMGS_PER_CORE):
                # groups 2/3's eager-L3 batches run as their L3 tiles land
                if i in (7, 11):
                    l3_batch(i // 4 + 1)
                gi, qi = GQ(i)
                gp, qp = GQ(i - 1)
                sts[i] = {}
                S1(i, gi, qi, z2s.pop(i), sts[i])
                S4(i - 1, gp, qp, sts[i - 1])
                S2(i, sts[i])
                S5(i - 1, sts.pop(i - 1))
                S3(i, gi, qi, sts[i])
            last = IMGS_PER_CORE - 1
            gl, ql = GQ(last)
            S4(last, gl, ql, sts[last])
            S5(last, sts.pop(last))

    split_excess_waits(nc)
    return nc


# ---------------------------------------------------------------------------
# Entry point
# ---------------------------------------------------------------------------
_NC_CACHE = []
_LAST_RESULT = []  # last BassKernelResults (exec_time_ns when BASS_TRACE=1)


def _axon_reset():
    try:
        import ctypes
        lib = ctypes.CDLL('/opt/axon/libaxon_pjrt.so')
        lib.axon_reset.restype = ctypes.c_int64
        lib.axon_reset()
    except Exception:
        pass


def kernel(yl, yh0, yh1, yh2, g0o, g1o, g0a, g0b, g1a, g1b):
    yl = np.ascontiguousarray(np.asarray(yl, np.float32).astype(np.float16))
    yh0 = np.ascontiguousarray(np.asarray(yh0, np.float32).astype(np.float16))
    yh1 = np.ascontiguousarray(np.asarray(yh1, np.float32).astype(np.float16))
    yh2 = np.ascontiguousarray(np.asarray(yh2, np.float32).astype(np.float16))
    assert yl.shape == (8, 16, 64, 64)

    mats = build_matrices(g0o, g1o, g0a, g0b, g1a, g1b)
    if not _NC_CACHE:
        _NC_CACHE.append(build_nc())
    nc = _NC_CACHE[0]

    big = pack_matrices(mats)
    in_maps = []
    for core in range(N_CORES):
        m = pack_inputs(yl[core], yh0[core], yh1[core], yh2[core])
        for k, v in m.items():
            assert v.shape == IN_SHAPES[k], (k, v.shape, IN_SHAPES[k])
        m["mats"] = big
        in_maps.append(m)

    try:
        res = run_bass_kernel_spmd(nc, in_maps, list(range(N_CORES)))
    except Exception as e:  # wedged exec unit: reset the axon device, retry
        if "UNAVAILABLE" not in str(e) and "unrecoverable" not in str(e):
            raise
        _axon_reset()
        res = run_bass_kernel_spmd(nc, in_maps, list(range(N_CORES)))
    _LAST_RESULT.clear()
    _LAST_RESULT.append(res)
    out = np.stack([res.results[i]["out"] for i in range(N_CORES)], axis=0)
    return np.ascontiguousarray(out.astype(np.float32))
